# revision 49
# baseline (speedup 1.0000x reference)
"""Trainium2 Bass kernel for nn_AttentionBlock (GroupNorm + 8-head self-attention + residual).

Full inputs in, full output out. Sharding: data-parallel over batch across the
8 NeuronCores (16 batches -> 2 per core), weights replicated, no collectives.

Layout strategy (per core, per batch; C=512 channels, S=1024 tokens):
  - x and xhat live as [C, S] tiles (channels on partitions) so GroupNorm
    scale/bias are per-partition scalars.  Cross-partition group reductions
    (16 channels/group) and the broadcast back are tiny PE matmuls against
    one-hot group matrices.
  - Q^T, K^T computed as [qk_rows, S] (head-major rows); V as [S, 8*65] with a
    ones-column per head (row-sums of exp fall out of the P@V matmul).
  - scores are computed TRANSPOSED: scoresT[j, i] = k_j . q_i so that the
    softmax reduction (over j) aligns with the matmul contraction axis and no
    transposes are ever needed.  exp() runs on ScalarE straight out of PSUM.
  - P@V gives resU^T [65, S] (row 64 = softmax denominators); normalization is
    a reciprocal of the sums row + gpsimd partition_broadcast + one DVE mul.
  - out-projection consumes resT directly; residual-add fused in the epilogue.
Attention/projection matmuls run in bf16 (fp32 PSUM accumulation); groupnorm
statistics stay fp32.  The softmax max-subtraction is skipped: scores are
~N(0,1) by construction (standardized activations, 1/sqrt(dk) folded into the
Q weights host-side), so exp() stays comfortably in fp32 range.
"""

import numpy as np
import ml_dtypes

import concourse.bacc as bacc
import concourse.tile as tile
from concourse import mybir
from concourse.bass_utils import run_bass_kernel_spmd

N_CORES = 8
B, C, H, W = 16, 512, 32, 32
S = H * W                      # 1024
BL = B // N_CORES              # 2 batches per core
NH, DK = 8, 64
NG = 32                        # groupnorm groups
GSZ = C // NG                  # 16 channels per group
EPS = 1e-5
F32 = mybir.dt.float32
BF16 = mybir.dt.bfloat16
AF = mybir.ActivationFunctionType
OP = mybir.AluOpType
NPBF16 = ml_dtypes.bfloat16

# test.py can flip these; results stashed in LAST.
TRACE = False
RECIP_MODE = "approx_sbuf"  # "approx_sbuf" | "plain"
LAST = {}


def _build(has_bqk, has_bv, has_outb, debug=False):
    nc = bacc.Bacc()
    dbg = {}
    if debug:
        for nm, shp in (
            ("dbg_xh", [C, S]),
            ("dbg_qt", [C, S]),
            ("dbg_kt", [C, S]),
            ("dbg_v", [8, 128, NH * 65]),
            ("dbg_ex", [128, S]),
            ("dbg_pvt", [65, S]),
            ("dbg_rrow", [1, S]),
            ("dbg_rbt", [64, S]),
            ("dbg_rt", [C, S]),
        ):
            dbg[nm] = nc.dram_tensor(nm, shp, F32, kind="ExternalOutput")

    x_d = nc.dram_tensor("x", [BL, C, S], F32, kind="ExternalInput")
    wqt_d = nc.dram_tensor("wqt", [C, C], BF16, kind="ExternalInput")   # [c_in, q_row]
    wkt_d = nc.dram_tensor("wkt", [C, C], BF16, kind="ExternalInput")
    wvt_d = nc.dram_tensor("wvt", [C, C], BF16, kind="ExternalInput")
    wot_d = nc.dram_tensor("wot", [C, C], BF16, kind="ExternalInput")   # [d_out, c_out]
    g_d = nc.dram_tensor("gmat", [128, 8], F32, kind="ExternalInput")
    gt_d = nc.dram_tensor("gtmat", [8, 128], F32, kind="ExternalInput")
    zpad_d = nc.dram_tensor("zpad", [64, S], BF16, kind="ExternalInput")
    bqk_d = (
        nc.dram_tensor("bqk", [128, 8], F32, kind="ExternalInput") if has_bqk else None
    )
    bv_d = nc.dram_tensor("bv", [1, C], BF16, kind="ExternalInput") if has_bv else None
    outb_d = (
        nc.dram_tensor("outb", [128, 4], F32, kind="ExternalInput") if has_outb else None
    )
    out_d = nc.dram_tensor("out", [BL, C, S], F32, kind="ExternalOutput")

    with tile.TileContext(nc) as tc:
        with (
            tc.tile_pool(name="const", bufs=1) as const,
            tc.tile_pool(name="px", bufs=4) as px,
            tc.tile_pool(name="pxe", bufs=4) as pxe,
            tc.tile_pool(name="pgn", bufs=4) as pgn,
            tc.tile_pool(name="pxh", bufs=8) as pxh,
            tc.tile_pool(name="pqt", bufs=8) as pqt,
            tc.tile_pool(name="pkt", bufs=16) as pkt,
            tc.tile_pool(name="pv", bufs=16) as pvp,
            tc.tile_pool(name="pexp", bufs=9) as pexp,
            tc.tile_pool(name="prec", bufs=3) as prec,
            tc.tile_pool(name="prt", bufs=8) as prt,
            tc.tile_pool(name="pout", bufs=3) as pout,
            tc.tile_pool(name="pps", bufs=2, space="PSUM") as pps,
            tc.tile_pool(name="psc", bufs=2, space="PSUM") as psc,
            tc.tile_pool(name="ppv", bufs=2, space="PSUM") as ppv,
        ):
            # ---- batch-0 x first (groupnorm needs it before weights)
            xt0 = []
            for cb in range(4):
                t = px.tile([128, S], F32, tag="x", name=f"x0_{cb}")
                nc.sync.dma_start(out=t, in_=x_d[0, cb * 128 : (cb + 1) * 128, :])
                xt0.append(t)

            # ---- constants into SBUF (small groupnorm mats first)
            g_sb = const.tile([128, 8], F32, tag="g")
            nc.sync.dma_start(out=g_sb, in_=g_d[:, :])
            gt_sb = const.tile([8, 128], F32, tag="gt")
            nc.sync.dma_start(out=gt_sb, in_=gt_d[:, :])
            wq_sb, wk_sb, wv_sb, wo_sb = [], [], [], []
            for nm, lst, src in (
                ("q", wq_sb, wqt_d),
                ("k", wk_sb, wkt_d),
                ("v", wv_sb, wvt_d),
                ("o", wo_sb, wot_d),
            ):
                for cb in range(4):
                    t = const.tile([128, C], BF16, tag=f"w_{nm}_{cb}")
                    nc.sync.dma_start(out=t, in_=src[cb * 128 : (cb + 1) * 128, :])
                    lst.append(t)
            eps_sb = const.tile([128, 1], F32, tag="eps")
            nc.vector.memset(eps_sb, EPS)
            if has_bqk:
                bqk_sb = const.tile([128, 8], F32, tag="bqk")
                nc.sync.dma_start(out=bqk_sb, in_=bqk_d[:, :])
            if has_bv:
                bv_sb = const.tile([1, C], BF16, tag="bv")
                nc.sync.dma_start(out=bv_sb, in_=bv_d[:, :])
                ones_sb = const.tile([1, S], BF16, tag="ones")
                nc.vector.memset(ones_sb, 1.0)
            if has_outb:
                outb_sb = const.tile([128, 4], F32, tag="outb")
                nc.sync.dma_start(out=outb_sb, in_=outb_d[:, :])

            # ---- PE warm-up: ~5us of dense dummy matmuls during the
            # DMA/groupnorm-bound startup so the HAM clock gate opens before
            # the first real projection matmul arrives.
            warm_ps = pps.tile([8, 128], F32, tag="pp", name="warm_ps")
            for wi in range(12):
                nc.tensor.matmul(
                    out=warm_ps,
                    lhsT=g_sb,
                    rhs=xt0[0][:, 0:128],
                    start=True,
                    stop=True,
                )

            # ================= emission helpers =================
            def load_x(b):
                xt = []
                for cb in range(4):
                    t = px.tile([128, S], F32, tag="x", name=f"x{b}_{cb}")
                    nc.sync.dma_start(out=t, in_=x_d[b, cb * 128 : (cb + 1) * 128, :])
                    xt.append(t)
                return xt

            def gn_batch(b, xt, xh):
                # groupnorm -> xhat for all 4 channel blocks; rstd computed on
                # DVE only (reciprocal seed + 2 Newton rsqrt steps) so ScalarE
                # never loads a non-Exp activation table.
                pgall = pps.tile([8, 4, 2], F32, tag="pp")   # [group, cb, (mean,e2)]
                for cb in range(4):
                    st6 = pgn.tile([128, 2, 6], F32, tag="st6")
                    nc.vector.bn_stats(out=st6[:, 0, :], in_=xt[cb][:, 0:512])
                    nc.vector.bn_stats(out=st6[:, 1, :], in_=xt[cb][:, 512:1024])
                    mv = pgn.tile([128, 2], F32, tag="mv")
                    nc.vector.bn_aggr(out=mv, in_=st6)
                    me2 = pgn.tile([128, 2], F32, tag="me2")
                    nc.vector.tensor_copy(out=me2[:, 0:1], in_=mv[:, 0:1])
                    nc.vector.tensor_tensor(
                        out=me2[:, 1:2], in0=mv[:, 0:1], in1=mv[:, 0:1], op=OP.mult
                    )
                    nc.vector.tensor_tensor(
                        out=me2[:, 1:2], in0=me2[:, 1:2], in1=mv[:, 1:2], op=OP.add
                    )
                    nc.tensor.matmul(
                        out=pgall[:, cb, :], lhsT=g_sb, rhs=me2, start=True, stop=True
                    )
                # group stats for all blocks at once ([8, 4] tiles)
                gm = pgn.tile([8, 4], F32, tag="gm")
                z = pgn.tile([8, 4], F32, tag="z")
                t2 = pgn.tile([8, 4], F32, tag="t2")
                y = pgn.tile([8, 4], F32, tag="y")
                nc.vector.tensor_scalar(
                    out=gm, in0=pgall[:, :, 0], scalar1=1.0 / GSZ, scalar2=None,
                    op0=OP.mult,
                )
                nc.vector.tensor_scalar(
                    out=z, in0=pgall[:, :, 1], scalar1=1.0 / GSZ, scalar2=EPS,
                    op0=OP.mult, op1=OP.add,
                )
                nc.vector.tensor_tensor(out=t2, in0=gm, in1=gm, op=OP.mult)
                nc.vector.tensor_tensor(out=z, in0=z, in1=t2, op=OP.subtract)
                # rsqrt(z): y0 = 1/z, then y <- y*(1.5 - 0.5*z*y^2) twice
                nc.vector.reciprocal(out=y, in_=z)
                for _ in range(2):
                    nc.vector.tensor_tensor(out=t2, in0=z, in1=y, op=OP.mult)
                    nc.vector.tensor_tensor(out=t2, in0=t2, in1=y, op=OP.mult)
                    nc.vector.tensor_scalar(
                        out=t2, in0=t2, scalar1=-0.5, scalar2=1.5,
                        op0=OP.mult, op1=OP.add,
                    )
                    nc.vector.tensor_tensor(out=y, in0=y, in1=t2, op=OP.mult)
                gs2 = pgn.tile([8, 2, 4], F32, tag="gs2")   # [(mean,rstd), cb]
                nc.vector.tensor_copy(out=gs2[:, 0, :], in_=gm)
                nc.vector.tensor_copy(out=gs2[:, 1, :], in_=y)
                for cb in range(4):
                    pb = pps.tile([128, 2], F32, tag="pp")
                    nc.tensor.matmul(
                        out=pb, lhsT=gt_sb, rhs=gs2[:, :, cb], start=True, stop=True
                    )
                    t = pxh.tile([128, S], BF16, tag="xh", name=f"xh{b}_{cb}")
                    nc.vector.tensor_scalar(
                        out=t,
                        in0=xt[cb],
                        scalar1=pb[:, 0:1],
                        scalar2=pb[:, 1:2],
                        op0=OP.subtract,
                        op1=OP.mult,
                    )
                    xh.append(t)
                    if debug and b == 0:
                        nc.gpsimd.dma_start(
                            out=dbg["dbg_xh"][cb * 128 : (cb + 1) * 128, :], in_=t
                        )

            def v_group(b, xh, vt, st):
                # one [S-tile, NH, 65] V tile with ones column per head
                pv = pps.tile([128, 512], F32, tag="pp")
                for cb in range(4):
                    nc.tensor.matmul(
                        out=pv,
                        lhsT=xh[cb][:, st * 128 : (st + 1) * 128],
                        rhs=wv_sb[cb],
                        start=(cb == 0),
                        stop=(cb == 3 and not has_bv),
                    )
                if has_bv:
                    nc.tensor.matmul(
                        out=pv,
                        lhsT=ones_sb[:, st * 128 : (st + 1) * 128],
                        rhs=bv_sb,
                        start=False,
                        stop=True,
                    )
                t = pvp.tile([128, NH, 65], BF16, tag="v", name=f"v{b}_{st}")
                nc.vector.memset(t[:, :, 64:65], 1.0)
                nc.vector.tensor_copy(
                    out=t[:, :, 0:64], in_=pv.rearrange("p (h d) -> p h d", h=NH)
                )
                vt.append(t)
                if debug and b == 0:
                    nc.gpsimd.dma_start(
                        out=dbg["dbg_v"][st, :, :],
                        in_=t.rearrange("p h d -> p (h d)"),
                    )

            def attn_head(b, qt, kt, vt, rt, h, fill=None, fill_every=2):
                # scores transposed -> exp -> P@V halves -> normalized resT rows
                # `fill` emits one unit of independent PE work per jb so the
                # Tensor engine stays dense while ScalarE works through exps.
                hp, off = h // 2, (h % 2) * 64
                ex_tiles = [None] * 8
                for jb in range(8):
                    ps = psc.tile([128, S], F32, tag="ps")
                    for sc in range(2):
                        cols = slice(sc * 512, (sc + 1) * 512)
                        nc.tensor.matmul(
                            out=ps[:, cols],
                            lhsT=kt[h][:, jb * 128 : (jb + 1) * 128],
                            rhs=qt[hp][:, cols],
                            start=True,
                            stop=True,
                        )
                    ex = pexp.tile([128, S], BF16, tag="ex")
                    nc.scalar.activation(out=ex, in_=ps, func=AF.Exp)
                    ex_tiles[jb] = ex
                    if fill is not None and jb % fill_every == fill_every - 1:
                        fill()
                    if debug and b == 0 and h == 0 and jb == 0:
                        nc.gpsimd.dma_start(out=dbg["dbg_ex"][:, :], in_=ex)

                # P@V: jb-major so both column halves reuse each weight load
                pvts = [ppv.tile([65, 512], F32, tag="ppvt", name=f"pvt{i}") for i in range(2)]
                for jb in range(8):
                    for sc in range(2):
                        nc.tensor.matmul(
                            out=pvts[sc],
                            lhsT=vt[jb][:, h, :],
                            rhs=ex_tiles[jb][:, sc * 512 : (sc + 1) * 512],
                            start=(jb == 0),
                            stop=(jb == 7),
                        )
                for sc in range(2):
                    pvt = pvts[sc]
                    if debug and b == 0 and h == 0:
                        dump = pout.tile([65, 512], F32, tag="dump", name=f"dmp{sc}")
                        nc.vector.tensor_copy(out=dump, in_=pvt)
                        nc.gpsimd.dma_start(
                            out=dbg["dbg_pvt"][:, sc * 512 : (sc + 1) * 512],
                            in_=dump,
                        )
                    # normalize: resT[head rows] = resU / rowsum
                    rrow = prec.tile([1, 512], F32, tag="rr")
                    if RECIP_MODE == "approx_sbuf":
                        stage = prec.tile([1, 512], F32, tag="st")
                        nc.vector.tensor_copy(out=stage, in_=pvt[64:65, :])
                        nc.vector.reciprocal_approx_fast(out=rrow, in_=stage)
                    else:
                        nc.vector.reciprocal(out=rrow, in_=pvt[64:65, :])
                    rbt = prec.tile([64, 512], F32, tag="rb")
                    nc.gpsimd.partition_broadcast(rbt, rrow)
                    nc.vector.tensor_tensor(
                        out=rt[hp][off : off + 64, sc * 512 : (sc + 1) * 512],
                        in0=pvt[0:64, :],
                        in1=rbt,
                        op=OP.mult,
                    )
                    if debug and b == 0 and h == 0:
                        nc.gpsimd.dma_start(
                            out=dbg["dbg_rrow"][:, sc * 512 : (sc + 1) * 512],
                            in_=rrow,
                        )
                        nc.gpsimd.dma_start(
                            out=dbg["dbg_rbt"][:, sc * 512 : (sc + 1) * 512],
                            in_=rbt,
                        )

            def drain(wl, n):
                for _ in range(min(n, len(wl))):
                    wl.pop(0)()

            def qk_units(b, xh, dst, w_sb, boff, rb):
                # one projection psum row-block.  For Q (boff==0): a single
                # head-pair tile.  For K (boff==4): two per-head tiles with the
                # other head's 64 rows zeroed, so the scores matmul can run a
                # full-K (128-partition) contraction at full SBUF stream rate.
                holder = {}

                def half(sc):
                    is_q = boff == 0
                    if "t" not in holder:
                        if is_q:
                            tq = pqt.tile(
                                [128, S], BF16, tag="qk", name=f"q{b}_{rb}"
                            )
                            holder["t"] = (tq,)
                            dst.append(tq)
                        else:
                            te = pkt.tile(
                                [128, S], BF16, tag="qk", name=f"k{b}_{rb}e"
                            )
                            to = pkt.tile(
                                [128, S], BF16, tag="qk", name=f"k{b}_{rb}o"
                            )
                            nc.sync.dma_start(out=te[64:128, :], in_=zpad_d[:, :])
                            nc.sync.dma_start(out=to[0:64, :], in_=zpad_d[:, :])
                            holder["t"] = (te, to)
                            dst.extend([te, to])
                    tiles = holder["t"]
                    pq = pps.tile([128, 512], F32, tag="pp")
                    for cb in range(4):
                        nc.tensor.matmul(
                            out=pq,
                            lhsT=w_sb[cb][:, rb * 128 : (rb + 1) * 128],
                            rhs=xh[cb][:, sc * 512 : (sc + 1) * 512],
                            start=(cb == 0),
                            stop=(cb == 3),
                        )
                    cols = slice(sc * 512, (sc + 1) * 512)
                    if is_q:
                        if has_bqk:
                            nc.vector.tensor_scalar_add(
                                out=tiles[0][:, cols],
                                in0=pq,
                                scalar1=bqk_sb[:, rb : rb + 1],
                            )
                        else:
                            nc.vector.tensor_copy(out=tiles[0][:, cols], in_=pq)
                    else:
                        for t, prng in (
                            (tiles[0], slice(0, 64)),
                            (tiles[1], slice(64, 128)),
                        ):
                            if has_bqk:
                                nc.vector.tensor_scalar_add(
                                    out=t[prng, cols],
                                    in0=pq[prng, :],
                                    scalar1=bqk_sb[prng, 4 + rb : 5 + rb],
                                )
                            else:
                                nc.vector.tensor_copy(out=t[prng, cols], in_=pq[prng, :])
                    if debug and b == 0 and sc == 1:
                        if is_q:
                            nc.gpsimd.dma_start(
                                out=dbg["dbg_qt"][rb * 128 : (rb + 1) * 128, :],
                                in_=tiles[0],
                            )
                        else:
                            nc.gpsimd.dma_start(
                                out=dbg["dbg_kt"][rb * 128 : rb * 128 + 64, :],
                                in_=tiles[0][0:64, :],
                            )
                            nc.gpsimd.dma_start(
                                out=dbg["dbg_kt"][rb * 128 + 64 : (rb + 1) * 128, :],
                                in_=tiles[1][64:128, :],
                            )

                return [lambda: half(0), lambda: half(1)]

            def epi_units(b, rt, cb):
                # epi_block split into two 512-column half-units (DMA on 2nd);
                # the residual x slice is re-loaded from DRAM.
                holder = {}

                def half(sc):
                    if "t" not in holder:
                        holder["t"] = pout.tile(
                            [128, S], F32, tag="ot", name=f"ot{b}_{cb}"
                        )
                    ot = holder["t"]
                    xre = pxe.tile([128, 512], F32, tag="xe")
                    nc.sync.dma_start(
                        out=xre,
                        in_=x_d[b, cb * 128 : (cb + 1) * 128, sc * 512 : (sc + 1) * 512],
                    )
                    po = pps.tile([128, 512], F32, tag="pp")
                    for db in range(4):
                        nc.tensor.matmul(
                            out=po,
                            lhsT=wo_sb[db][:, cb * 128 : (cb + 1) * 128],
                            rhs=rt[db][:, sc * 512 : (sc + 1) * 512],
                            start=(db == 0),
                            stop=(db == 3),
                        )
                    dst_ap = ot[:, sc * 512 : (sc + 1) * 512]
                    if has_outb:
                        nc.vector.scalar_tensor_tensor(
                            out=dst_ap,
                            in0=po,
                            scalar=outb_sb[:, cb : cb + 1],
                            in1=xre,
                            op0=OP.add,
                            op1=OP.add,
                        )
                    else:
                        nc.vector.tensor_tensor(out=dst_ap, in0=po, in1=xre, op=OP.add)
                    if sc == 1:
                        nc.sync.dma_start(
                            out=out_d[b, cb * 128 : (cb + 1) * 128, :], in_=ot
                        )

                return [lambda: half(0), lambda: half(1)]

            # ================= schedule =================
            # batch 0 prep emitted directly; batch 1 prep + batch 0 epilogue
            # are emitted one psum-group at a time inside the attention loops
            # (fill callback per jb) so the Tensor engine always has dense
            # independent work while ScalarE works through the exps.
            xt1 = load_x(1)
            xh0, qt0, kt0, vt0 = [], [], [], []
            gn_batch(0, xt0, xh0)
            # V first, then only the rb0 row-blocks of Q/K: attention(0) can
            # then start at head 0 while the remaining row-blocks stream in as
            # fill work inside the head loop.
            for st in range(8):
                v_group(0, xh0, vt0, st)
            for u in qk_units(0, xh0, qt0, wq_sb, 0, 0):
                u()
            for u in qk_units(0, xh0, kt0, wk_sb, 4, 0):
                u()

            xh1, qt1, kt1, vt1 = [], [], [], []
            gn_batch(1, xt1, xh1)
            work1 = []
            for rb in range(1, 4):
                work1.extend(qk_units(0, xh0, qt0, wq_sb, 0, rb))
                work1.extend(qk_units(0, xh0, kt0, wk_sb, 4, rb))
            for rb in range(2):
                work1.extend(qk_units(1, xh1, qt1, wq_sb, 0, rb))
                work1.extend(qk_units(1, xh1, kt1, wk_sb, 4, rb))
            for st in range(8):
                work1.append(lambda st=st: v_group(1, xh1, vt1, st))

            rt0 = [prt.tile([128, S], BF16, tag="rt", name=f"rt0_{i}") for i in range(4)]
            for h in range(NH):
                attn_head(0, qt0, kt0, vt0, rt0, h, fill=lambda: drain(work1, 1))
            drain(work1, len(work1))
            if debug:
                for hp in range(4):
                    nc.gpsimd.dma_start(
                        out=dbg["dbg_rt"][hp * 128 : (hp + 1) * 128, :], in_=rt0[hp]
                    )

            # batch-1 attention: deferred qk row-blocks (heads 4-7) first, then
            # batch-0 epilogue halves, spread evenly (fill every 4th jb)
            work2 = []
            for rb in range(2, 4):
                work2.extend(qk_units(1, xh1, qt1, wq_sb, 0, rb))
                work2.extend(qk_units(1, xh1, kt1, wk_sb, 4, rb))
            for cb in range(4):
                work2.extend(epi_units(0, rt0, cb))
            rt1 = [prt.tile([128, S], BF16, tag="rt", name=f"rt1_{i}") for i in range(4)]
            for h in range(NH):
                attn_head(
                    1, qt1, kt1, vt1, rt1, h,
                    fill=lambda: drain(work2, 1), fill_every=4,
                )
            drain(work2, len(work2))
            for cb in range(4):
                for u in epi_units(1, rt1, cb):
                    u()

    nc.finalize()
    return nc


def kernel(**inputs):
    x = np.asarray(inputs["x"], np.float32)
    norm_w = np.asarray(inputs["norm_w"], np.float64)
    norm_b = np.asarray(inputs["norm_b"], np.float64)
    proj_w = np.asarray(inputs["proj_w"], np.float64)
    proj_b = np.asarray(inputs["proj_b"], np.float64)
    out_w = np.asarray(inputs["out_w"], np.float32)
    out_b = np.asarray(inputs["out_b"], np.float32)

    # split qkv rows (row = h*192 + t*64 + d, t in {q,k,v}) into head-major mats
    pw = proj_w.reshape(NH, 3, DK, C)
    pb = proj_b.reshape(NH, 3, DK)
    mats, biases = [], []
    for t in range(3):
        wm = pw[:, t].reshape(NH * DK, C)
        bv = pb[:, t].reshape(NH * DK)
        # fold groupnorm affine: y = xhat*nw + nb  =>  W@y + b = (W*nw)@xhat + (W@nb + b)
        mats.append(wm * norm_w[None, :])
        biases.append(bv + wm @ norm_b)
    wq, wk, wv = mats
    bq, bk, bv = biases
    scale = DK ** -0.5
    wq = wq * scale
    bq = bq * scale

    wqT = np.ascontiguousarray(wq.T).astype(NPBF16)
    wkT = np.ascontiguousarray(wk.T).astype(NPBF16)
    wvT = np.ascontiguousarray(wv.T).astype(NPBF16)
    woT = np.ascontiguousarray(out_w.T).astype(NPBF16)

    G = np.zeros((128, 8), np.float32)
    G[np.arange(128), np.arange(128) // GSZ] = 1.0
    GT = np.ascontiguousarray(G.T)
    ZPAD = np.zeros((64, S), NPBF16)

    has_bqk = bool(np.any(bq) or np.any(bk))
    has_bv = bool(np.any(bv))
    has_outb = bool(np.any(out_b))

    bqk = np.zeros((128, 8), np.float32)
    bqk[:, 0:4] = bq.reshape(4, 128).T
    bqk[:, 4:8] = bk.reshape(4, 128).T
    outb128 = np.ascontiguousarray(out_b.reshape(4, 128).T)

    nc = _build(has_bqk, has_bv, has_outb)

    xr = x.reshape(B, C, S)
    in_maps = []
    for c in range(N_CORES):
        m = {
            "x": np.ascontiguousarray(xr[c * BL : (c + 1) * BL]),
            "wqt": wqT,
            "wkt": wkT,
            "wvt": wvT,
            "wot": woT,
            "gmat": G,
            "gtmat": GT,
            "zpad": ZPAD,
        }
        if has_bqk:
            m["bqk"] = bqk
        if has_bv:
            m["bv"] = np.ascontiguousarray(bv.reshape(1, C)).astype(NPBF16)
        if has_outb:
            m["outb"] = outb128
        in_maps.append(m)

    # guard: bass_utils imports antenv.axon_hooks when tracing is requested
    # (e.g. via BASS_TRACE env); provide a no-op module if the image lacks it.
    try:
        import antenv.axon_hooks  # noqa: F401
    except ImportError:
        import sys
        import types

        import antenv

        _m = types.ModuleType("antenv.axon_hooks")
        _m._hook = None
        _m.set_axon_ntff_profile_hook = lambda h: setattr(_m, "_hook", h)
        _m.get_axon_ntff_profile_hook = lambda: _m._hook
        sys.modules["antenv.axon_hooks"] = _m
        antenv.axon_hooks = _m

    res = None
    for attempt in range(3):
        try:
            res = run_bass_kernel_spmd(
                nc, in_maps, core_ids=list(range(N_CORES)), trace=TRACE
            )
            break
        except Exception:
            # transient NRT_EXEC_UNIT_UNRECOVERABLE-style device hiccups
            # clear on retry; re-raise on the final attempt
            if attempt == 2:
                raise
    LAST["exec_time_ns"] = res.exec_time_ns
    LAST["mean_exec_time_ns"] = res.mean_exec_time_ns
    LAST["result"] = res

    out = np.concatenate([res.results[c]["out"] for c in range(N_CORES)], axis=0)
    return np.ascontiguousarray(out.reshape(B, C, H, W).astype(np.float32))


# revision 50
# speedup vs baseline: 1.0089x; 1.0089x over previous
"""Trainium2 Bass kernel for nn_AttentionBlock (GroupNorm + 8-head self-attention + residual).

Full inputs in, full output out. Sharding: data-parallel over batch across the
8 NeuronCores (16 batches -> 2 per core), weights replicated, no collectives.

Layout strategy (per core, per batch; C=512 channels, S=1024 tokens):
  - x and xhat live as [C, S] tiles (channels on partitions) so GroupNorm
    scale/bias are per-partition scalars.  Cross-partition group reductions
    (16 channels/group) and the broadcast back are tiny PE matmuls against
    one-hot group matrices.
  - Q^T, K^T computed as [qk_rows, S] (head-major rows); V as [S, 8*65] with a
    ones-column per head (row-sums of exp fall out of the P@V matmul).
  - scores are computed TRANSPOSED: scoresT[j, i] = k_j . q_i so that the
    softmax reduction (over j) aligns with the matmul contraction axis and no
    transposes are ever needed.  exp() runs on ScalarE straight out of PSUM.
  - P@V gives resU^T [65, S] (row 64 = softmax denominators); normalization is
    a reciprocal of the sums row + gpsimd partition_broadcast + one DVE mul.
  - out-projection consumes resT directly; residual-add fused in the epilogue.
Attention/projection matmuls run in bf16 (fp32 PSUM accumulation); groupnorm
statistics stay fp32.  The softmax max-subtraction is skipped: scores are
~N(0,1) by construction (standardized activations, 1/sqrt(dk) folded into the
Q weights host-side), so exp() stays comfortably in fp32 range.
"""

import numpy as np
import ml_dtypes

import concourse.bacc as bacc
import concourse.tile as tile
from concourse import mybir
from concourse.bass_utils import run_bass_kernel_spmd

N_CORES = 8
B, C, H, W = 16, 512, 32, 32
S = H * W                      # 1024
BL = B // N_CORES              # 2 batches per core
NH, DK = 8, 64
NG = 32                        # groupnorm groups
GSZ = C // NG                  # 16 channels per group
EPS = 1e-5
F32 = mybir.dt.float32
BF16 = mybir.dt.bfloat16
AF = mybir.ActivationFunctionType
OP = mybir.AluOpType
NPBF16 = ml_dtypes.bfloat16

# test.py can flip these; results stashed in LAST.
TRACE = False
RECIP_MODE = "approx_sbuf"  # "approx_sbuf" | "plain"
LAST = {}


def _build(has_bqk, has_bv, has_outb, debug=False):
    nc = bacc.Bacc()
    dbg = {}
    if debug:
        for nm, shp in (
            ("dbg_xh", [C, S]),
            ("dbg_qt", [C, S]),
            ("dbg_kt", [C, S]),
            ("dbg_v", [8, 128, NH * 65]),
            ("dbg_ex", [128, S]),
            ("dbg_pvt", [65, S]),
            ("dbg_rrow", [1, S]),
            ("dbg_rbt", [64, S]),
            ("dbg_rt", [C, S]),
        ):
            dbg[nm] = nc.dram_tensor(nm, shp, F32, kind="ExternalOutput")

    x_d = nc.dram_tensor("x", [BL, C, S], F32, kind="ExternalInput")
    wqt_d = nc.dram_tensor("wqt", [C, C], BF16, kind="ExternalInput")   # [c_in, q_row]
    wkt_d = nc.dram_tensor("wkt", [C, C], BF16, kind="ExternalInput")
    wvt_d = nc.dram_tensor("wvt", [C, C], BF16, kind="ExternalInput")
    wot_d = nc.dram_tensor("wot", [C, C], BF16, kind="ExternalInput")   # [d_out, c_out]
    g_d = nc.dram_tensor("gmat", [128, 8], F32, kind="ExternalInput")
    gt_d = nc.dram_tensor("gtmat", [8, 128], F32, kind="ExternalInput")
    zpad_d = nc.dram_tensor("zpad", [64, S], BF16, kind="ExternalInput")
    bqk_d = (
        nc.dram_tensor("bqk", [128, 8], F32, kind="ExternalInput") if has_bqk else None
    )
    bv_d = nc.dram_tensor("bv", [1, C], BF16, kind="ExternalInput") if has_bv else None
    outb_d = (
        nc.dram_tensor("outb", [128, 4], F32, kind="ExternalInput") if has_outb else None
    )
    out_d = nc.dram_tensor("out", [BL, C, S], F32, kind="ExternalOutput")

    with tile.TileContext(nc) as tc:
        with (
            tc.tile_pool(name="const", bufs=1) as const,
            tc.tile_pool(name="px", bufs=4) as px,
            tc.tile_pool(name="pxe", bufs=4) as pxe,
            tc.tile_pool(name="pgn", bufs=4) as pgn,
            tc.tile_pool(name="pxh", bufs=8) as pxh,
            tc.tile_pool(name="pqt", bufs=8) as pqt,
            tc.tile_pool(name="pkt", bufs=16) as pkt,
            tc.tile_pool(name="pv", bufs=16) as pvp,
            tc.tile_pool(name="pexp", bufs=9) as pexp,
            tc.tile_pool(name="prec", bufs=3) as prec,
            tc.tile_pool(name="prt", bufs=8) as prt,
            tc.tile_pool(name="pout", bufs=3) as pout,
            tc.tile_pool(name="pps", bufs=2, space="PSUM") as pps,
            tc.tile_pool(name="psc", bufs=2, space="PSUM") as psc,
            tc.tile_pool(name="ppv", bufs=2, space="PSUM") as ppv,
        ):
            # ---- batch-0 x first (groupnorm needs it before weights)
            xt0 = []
            for cb in range(4):
                t = px.tile([128, S], F32, tag="x", name=f"x0_{cb}")
                nc.sync.dma_start(out=t, in_=x_d[0, cb * 128 : (cb + 1) * 128, :])
                xt0.append(t)

            # ---- constants into SBUF (small groupnorm mats first)
            g_sb = const.tile([128, 8], F32, tag="g")
            nc.sync.dma_start(out=g_sb, in_=g_d[:, :])
            gt_sb = const.tile([8, 128], F32, tag="gt")
            nc.sync.dma_start(out=gt_sb, in_=gt_d[:, :])
            wq_sb, wk_sb, wv_sb, wo_sb = [], [], [], []
            for nm, lst, src in (
                ("q", wq_sb, wqt_d),
                ("k", wk_sb, wkt_d),
                ("v", wv_sb, wvt_d),
                ("o", wo_sb, wot_d),
            ):
                for cb in range(4):
                    t = const.tile([128, C], BF16, tag=f"w_{nm}_{cb}")
                    nc.sync.dma_start(out=t, in_=src[cb * 128 : (cb + 1) * 128, :])
                    lst.append(t)
            eps_sb = const.tile([128, 1], F32, tag="eps")
            nc.vector.memset(eps_sb, EPS)
            if has_bqk:
                bqk_sb = const.tile([128, 8], F32, tag="bqk")
                nc.sync.dma_start(out=bqk_sb, in_=bqk_d[:, :])
            if has_bv:
                bv_sb = const.tile([1, C], BF16, tag="bv")
                nc.sync.dma_start(out=bv_sb, in_=bv_d[:, :])
                ones_sb = const.tile([1, S], BF16, tag="ones")
                nc.vector.memset(ones_sb, 1.0)
            if has_outb:
                outb_sb = const.tile([128, 4], F32, tag="outb")
                nc.sync.dma_start(out=outb_sb, in_=outb_d[:, :])

            # ---- PE warm-up: dense dummy matmuls during the DMA/groupnorm
            # bound startup so the HAM clock gate opens before the first real
            # projection matmul arrives.  Fed from a memset tile so the burst
            # needs no DMA and starts immediately.
            warm_sb = const.tile([128, 128], BF16, tag="warm")
            nc.vector.memset(warm_sb, 0.5)
            warm_ps = pps.tile([128, 128], F32, tag="pp", name="warm_ps")
            for wi in range(32):
                nc.tensor.matmul(
                    out=warm_ps,
                    lhsT=warm_sb,
                    rhs=warm_sb,
                    start=True,
                    stop=True,
                )

            # ================= emission helpers =================
            def load_x(b):
                xt = []
                for cb in range(4):
                    t = px.tile([128, S], F32, tag="x", name=f"x{b}_{cb}")
                    nc.sync.dma_start(out=t, in_=x_d[b, cb * 128 : (cb + 1) * 128, :])
                    xt.append(t)
                return xt

            def gn_batch(b, xt, xh):
                # groupnorm -> xhat for all 4 channel blocks; rstd computed on
                # DVE only (reciprocal seed + 2 Newton rsqrt steps) so ScalarE
                # never loads a non-Exp activation table.
                pgall = pps.tile([8, 4, 2], F32, tag="pp")   # [group, cb, (mean,e2)]
                for cb in range(4):
                    st6 = pgn.tile([128, 2, 6], F32, tag="st6")
                    nc.vector.bn_stats(out=st6[:, 0, :], in_=xt[cb][:, 0:512])
                    nc.vector.bn_stats(out=st6[:, 1, :], in_=xt[cb][:, 512:1024])
                    mv = pgn.tile([128, 2], F32, tag="mv")
                    nc.vector.bn_aggr(out=mv, in_=st6)
                    me2 = pgn.tile([128, 2], F32, tag="me2")
                    nc.vector.tensor_copy(out=me2[:, 0:1], in_=mv[:, 0:1])
                    nc.vector.tensor_tensor(
                        out=me2[:, 1:2], in0=mv[:, 0:1], in1=mv[:, 0:1], op=OP.mult
                    )
                    nc.vector.tensor_tensor(
                        out=me2[:, 1:2], in0=me2[:, 1:2], in1=mv[:, 1:2], op=OP.add
                    )
                    nc.tensor.matmul(
                        out=pgall[:, cb, :], lhsT=g_sb, rhs=me2, start=True, stop=True
                    )
                # group stats for all blocks at once ([8, 4] tiles)
                gm = pgn.tile([8, 4], F32, tag="gm")
                z = pgn.tile([8, 4], F32, tag="z")
                t2 = pgn.tile([8, 4], F32, tag="t2")
                y = pgn.tile([8, 4], F32, tag="y")
                nc.vector.tensor_scalar(
                    out=gm, in0=pgall[:, :, 0], scalar1=1.0 / GSZ, scalar2=None,
                    op0=OP.mult,
                )
                nc.vector.tensor_scalar(
                    out=z, in0=pgall[:, :, 1], scalar1=1.0 / GSZ, scalar2=EPS,
                    op0=OP.mult, op1=OP.add,
                )
                nc.vector.tensor_tensor(out=t2, in0=gm, in1=gm, op=OP.mult)
                nc.vector.tensor_tensor(out=z, in0=z, in1=t2, op=OP.subtract)
                # rsqrt(z): y0 = 1/z, then y <- y*(1.5 - 0.5*z*y^2) twice
                nc.vector.reciprocal(out=y, in_=z)
                for _ in range(2):
                    nc.vector.tensor_tensor(out=t2, in0=z, in1=y, op=OP.mult)
                    nc.vector.tensor_tensor(out=t2, in0=t2, in1=y, op=OP.mult)
                    nc.vector.tensor_scalar(
                        out=t2, in0=t2, scalar1=-0.5, scalar2=1.5,
                        op0=OP.mult, op1=OP.add,
                    )
                    nc.vector.tensor_tensor(out=y, in0=y, in1=t2, op=OP.mult)
                gs2 = pgn.tile([8, 2, 4], F32, tag="gs2")   # [(mean,rstd), cb]
                nc.vector.tensor_copy(out=gs2[:, 0, :], in_=gm)
                nc.vector.tensor_copy(out=gs2[:, 1, :], in_=y)
                for cb in range(4):
                    pb = pps.tile([128, 2], F32, tag="pp")
                    nc.tensor.matmul(
                        out=pb, lhsT=gt_sb, rhs=gs2[:, :, cb], start=True, stop=True
                    )
                    t = pxh.tile([128, S], BF16, tag="xh", name=f"xh{b}_{cb}")
                    nc.vector.tensor_scalar(
                        out=t,
                        in0=xt[cb],
                        scalar1=pb[:, 0:1],
                        scalar2=pb[:, 1:2],
                        op0=OP.subtract,
                        op1=OP.mult,
                    )
                    xh.append(t)
                    if debug and b == 0:
                        nc.gpsimd.dma_start(
                            out=dbg["dbg_xh"][cb * 128 : (cb + 1) * 128, :], in_=t
                        )

            def v_group(b, xh, vt, st):
                # one [S-tile, NH, 65] V tile with ones column per head
                pv = pps.tile([128, 512], F32, tag="pp")
                for cb in range(4):
                    nc.tensor.matmul(
                        out=pv,
                        lhsT=xh[cb][:, st * 128 : (st + 1) * 128],
                        rhs=wv_sb[cb],
                        start=(cb == 0),
                        stop=(cb == 3 and not has_bv),
                    )
                if has_bv:
                    nc.tensor.matmul(
                        out=pv,
                        lhsT=ones_sb[:, st * 128 : (st + 1) * 128],
                        rhs=bv_sb,
                        start=False,
                        stop=True,
                    )
                t = pvp.tile([128, NH, 65], BF16, tag="v", name=f"v{b}_{st}")
                nc.vector.memset(t[:, :, 64:65], 1.0)
                nc.vector.tensor_copy(
                    out=t[:, :, 0:64], in_=pv.rearrange("p (h d) -> p h d", h=NH)
                )
                vt.append(t)
                if debug and b == 0:
                    nc.gpsimd.dma_start(
                        out=dbg["dbg_v"][st, :, :],
                        in_=t.rearrange("p h d -> p (h d)"),
                    )

            def attn_head(b, qt, kt, vt, rt, h, fill=None, fill_every=2):
                # scores transposed -> exp -> P@V halves -> normalized resT rows
                # `fill` emits one unit of independent PE work per jb so the
                # Tensor engine stays dense while ScalarE works through exps.
                hp, off = h // 2, (h % 2) * 64
                ex_tiles = [None] * 8
                for jb in range(8):
                    ps = psc.tile([128, S], F32, tag="ps")
                    for sc in range(2):
                        cols = slice(sc * 512, (sc + 1) * 512)
                        nc.tensor.matmul(
                            out=ps[:, cols],
                            lhsT=kt[h][:, jb * 128 : (jb + 1) * 128],
                            rhs=qt[hp][:, cols],
                            start=True,
                            stop=True,
                        )
                    ex = pexp.tile([128, S], BF16, tag="ex")
                    nc.scalar.activation(out=ex, in_=ps, func=AF.Exp)
                    ex_tiles[jb] = ex
                    if fill is not None and jb % fill_every == fill_every - 1:
                        fill()
                    if debug and b == 0 and h == 0 and jb == 0:
                        nc.gpsimd.dma_start(out=dbg["dbg_ex"][:, :], in_=ex)

                # P@V: jb-major so both column halves reuse each weight load
                pvts = [ppv.tile([65, 512], F32, tag="ppvt", name=f"pvt{i}") for i in range(2)]
                for jb in range(8):
                    for sc in range(2):
                        nc.tensor.matmul(
                            out=pvts[sc],
                            lhsT=vt[jb][:, h, :],
                            rhs=ex_tiles[jb][:, sc * 512 : (sc + 1) * 512],
                            start=(jb == 0),
                            stop=(jb == 7),
                        )
                for sc in range(2):
                    pvt = pvts[sc]
                    if debug and b == 0 and h == 0:
                        dump = pout.tile([65, 512], F32, tag="dump", name=f"dmp{sc}")
                        nc.vector.tensor_copy(out=dump, in_=pvt)
                        nc.gpsimd.dma_start(
                            out=dbg["dbg_pvt"][:, sc * 512 : (sc + 1) * 512],
                            in_=dump,
                        )
                    # normalize: resT[head rows] = resU / rowsum
                    rrow = prec.tile([1, 512], F32, tag="rr")
                    if RECIP_MODE == "approx_sbuf":
                        stage = prec.tile([1, 512], F32, tag="st")
                        nc.vector.tensor_copy(out=stage, in_=pvt[64:65, :])
                        nc.vector.reciprocal_approx_fast(out=rrow, in_=stage)
                    else:
                        nc.vector.reciprocal(out=rrow, in_=pvt[64:65, :])
                    rbt = prec.tile([64, 512], F32, tag="rb")
                    nc.gpsimd.partition_broadcast(rbt, rrow)
                    nc.vector.tensor_tensor(
                        out=rt[hp][off : off + 64, sc * 512 : (sc + 1) * 512],
                        in0=pvt[0:64, :],
                        in1=rbt,
                        op=OP.mult,
                    )
                    if debug and b == 0 and h == 0:
                        nc.gpsimd.dma_start(
                            out=dbg["dbg_rrow"][:, sc * 512 : (sc + 1) * 512],
                            in_=rrow,
                        )
                        nc.gpsimd.dma_start(
                            out=dbg["dbg_rbt"][:, sc * 512 : (sc + 1) * 512],
                            in_=rbt,
                        )

            def drain(wl, n):
                for _ in range(min(n, len(wl))):
                    wl.pop(0)()

            def qk_units(b, xh, dst, w_sb, boff, rb):
                # one projection psum row-block.  For Q (boff==0): a single
                # head-pair tile.  For K (boff==4): two per-head tiles with the
                # other head's 64 rows zeroed, so the scores matmul can run a
                # full-K (128-partition) contraction at full SBUF stream rate.
                holder = {}

                def half(sc):
                    is_q = boff == 0
                    if "t" not in holder:
                        if is_q:
                            tq = pqt.tile(
                                [128, S], BF16, tag="qk", name=f"q{b}_{rb}"
                            )
                            holder["t"] = (tq,)
                            dst.append(tq)
                        else:
                            te = pkt.tile(
                                [128, S], BF16, tag="qk", name=f"k{b}_{rb}e"
                            )
                            to = pkt.tile(
                                [128, S], BF16, tag="qk", name=f"k{b}_{rb}o"
                            )
                            nc.sync.dma_start(out=te[64:128, :], in_=zpad_d[:, :])
                            nc.sync.dma_start(out=to[0:64, :], in_=zpad_d[:, :])
                            holder["t"] = (te, to)
                            dst.extend([te, to])
                    tiles = holder["t"]
                    pq = pps.tile([128, 512], F32, tag="pp")
                    for cb in range(4):
                        nc.tensor.matmul(
                            out=pq,
                            lhsT=w_sb[cb][:, rb * 128 : (rb + 1) * 128],
                            rhs=xh[cb][:, sc * 512 : (sc + 1) * 512],
                            start=(cb == 0),
                            stop=(cb == 3),
                        )
                    cols = slice(sc * 512, (sc + 1) * 512)
                    if is_q:
                        if has_bqk:
                            nc.vector.tensor_scalar_add(
                                out=tiles[0][:, cols],
                                in0=pq,
                                scalar1=bqk_sb[:, rb : rb + 1],
                            )
                        else:
                            nc.vector.tensor_copy(out=tiles[0][:, cols], in_=pq)
                    else:
                        for t, prng in (
                            (tiles[0], slice(0, 64)),
                            (tiles[1], slice(64, 128)),
                        ):
                            if has_bqk:
                                nc.vector.tensor_scalar_add(
                                    out=t[prng, cols],
                                    in0=pq[prng, :],
                                    scalar1=bqk_sb[prng, 4 + rb : 5 + rb],
                                )
                            else:
                                nc.vector.tensor_copy(out=t[prng, cols], in_=pq[prng, :])
                    if debug and b == 0 and sc == 1:
                        if is_q:
                            nc.gpsimd.dma_start(
                                out=dbg["dbg_qt"][rb * 128 : (rb + 1) * 128, :],
                                in_=tiles[0],
                            )
                        else:
                            nc.gpsimd.dma_start(
                                out=dbg["dbg_kt"][rb * 128 : rb * 128 + 64, :],
                                in_=tiles[0][0:64, :],
                            )
                            nc.gpsimd.dma_start(
                                out=dbg["dbg_kt"][rb * 128 + 64 : (rb + 1) * 128, :],
                                in_=tiles[1][64:128, :],
                            )

                return [lambda: half(0), lambda: half(1)]

            def epi_units(b, rt, cb):
                # epi_block split into two 512-column half-units (DMA on 2nd);
                # the residual x slice is re-loaded from DRAM.
                holder = {}

                def half(sc):
                    if "t" not in holder:
                        holder["t"] = pout.tile(
                            [128, S], F32, tag="ot", name=f"ot{b}_{cb}"
                        )
                    ot = holder["t"]
                    xre = pxe.tile([128, 512], F32, tag="xe")
                    nc.sync.dma_start(
                        out=xre,
                        in_=x_d[b, cb * 128 : (cb + 1) * 128, sc * 512 : (sc + 1) * 512],
                    )
                    po = pps.tile([128, 512], F32, tag="pp")
                    for db in range(4):
                        nc.tensor.matmul(
                            out=po,
                            lhsT=wo_sb[db][:, cb * 128 : (cb + 1) * 128],
                            rhs=rt[db][:, sc * 512 : (sc + 1) * 512],
                            start=(db == 0),
                            stop=(db == 3),
                        )
                    dst_ap = ot[:, sc * 512 : (sc + 1) * 512]
                    if has_outb:
                        nc.vector.scalar_tensor_tensor(
                            out=dst_ap,
                            in0=po,
                            scalar=outb_sb[:, cb : cb + 1],
                            in1=xre,
                            op0=OP.add,
                            op1=OP.add,
                        )
                    else:
                        nc.vector.tensor_tensor(out=dst_ap, in0=po, in1=xre, op=OP.add)
                    if sc == 1:
                        nc.sync.dma_start(
                            out=out_d[b, cb * 128 : (cb + 1) * 128, :], in_=ot
                        )

                return [lambda: half(0), lambda: half(1)]

            # ================= schedule =================
            # batch 0 prep emitted directly; batch 1 prep + batch 0 epilogue
            # are emitted one psum-group at a time inside the attention loops
            # (fill callback per jb) so the Tensor engine always has dense
            # independent work while ScalarE works through the exps.
            xt1 = load_x(1)
            xh0, qt0, kt0, vt0 = [], [], [], []
            gn_batch(0, xt0, xh0)
            # bridge burst: keep the PE active between the warm-up and the
            # first projection group (gated on the first xhat tile)
            warm_ps2 = pps.tile([128, 512], F32, tag="pp", name="warm_ps2")
            for wi in range(8):
                nc.tensor.matmul(
                    out=warm_ps2,
                    lhsT=warm_sb,
                    rhs=xh0[0][:, 0:512],
                    start=True,
                    stop=True,
                )
            # V first, then only the rb0 row-blocks of Q/K: attention(0) can
            # then start at head 0 while the remaining row-blocks stream in as
            # fill work inside the head loop.
            for st in range(8):
                v_group(0, xh0, vt0, st)
            for u in qk_units(0, xh0, qt0, wq_sb, 0, 0):
                u()
            for u in qk_units(0, xh0, kt0, wk_sb, 4, 0):
                u()

            xh1, qt1, kt1, vt1 = [], [], [], []
            gn_batch(1, xt1, xh1)
            work1 = []
            for rb in range(1, 4):
                work1.extend(qk_units(0, xh0, qt0, wq_sb, 0, rb))
                work1.extend(qk_units(0, xh0, kt0, wk_sb, 4, rb))
            for rb in range(2):
                work1.extend(qk_units(1, xh1, qt1, wq_sb, 0, rb))
                work1.extend(qk_units(1, xh1, kt1, wk_sb, 4, rb))
            for st in range(8):
                work1.append(lambda st=st: v_group(1, xh1, vt1, st))

            rt0 = [prt.tile([128, S], BF16, tag="rt", name=f"rt0_{i}") for i in range(4)]
            for h in range(NH):
                attn_head(0, qt0, kt0, vt0, rt0, h, fill=lambda: drain(work1, 1))
            drain(work1, len(work1))
            if debug:
                for hp in range(4):
                    nc.gpsimd.dma_start(
                        out=dbg["dbg_rt"][hp * 128 : (hp + 1) * 128, :], in_=rt0[hp]
                    )

            # batch-1 attention: deferred qk row-blocks (heads 4-7) first, then
            # batch-0 epilogue halves, spread evenly (fill every 4th jb)
            work2 = []
            for rb in range(2, 4):
                work2.extend(qk_units(1, xh1, qt1, wq_sb, 0, rb))
                work2.extend(qk_units(1, xh1, kt1, wk_sb, 4, rb))
            for cb in range(4):
                work2.extend(epi_units(0, rt0, cb))
            rt1 = [prt.tile([128, S], BF16, tag="rt", name=f"rt1_{i}") for i in range(4)]
            for h in range(NH):
                attn_head(
                    1, qt1, kt1, vt1, rt1, h,
                    fill=lambda: drain(work2, 1), fill_every=4,
                )
            drain(work2, len(work2))
            for cb in range(4):
                for u in epi_units(1, rt1, cb):
                    u()

    nc.finalize()
    return nc


def kernel(**inputs):
    x = np.asarray(inputs["x"], np.float32)
    norm_w = np.asarray(inputs["norm_w"], np.float64)
    norm_b = np.asarray(inputs["norm_b"], np.float64)
    proj_w = np.asarray(inputs["proj_w"], np.float64)
    proj_b = np.asarray(inputs["proj_b"], np.float64)
    out_w = np.asarray(inputs["out_w"], np.float32)
    out_b = np.asarray(inputs["out_b"], np.float32)

    # split qkv rows (row = h*192 + t*64 + d, t in {q,k,v}) into head-major mats
    pw = proj_w.reshape(NH, 3, DK, C)
    pb = proj_b.reshape(NH, 3, DK)
    mats, biases = [], []
    for t in range(3):
        wm = pw[:, t].reshape(NH * DK, C)
        bv = pb[:, t].reshape(NH * DK)
        # fold groupnorm affine: y = xhat*nw + nb  =>  W@y + b = (W*nw)@xhat + (W@nb + b)
        mats.append(wm * norm_w[None, :])
        biases.append(bv + wm @ norm_b)
    wq, wk, wv = mats
    bq, bk, bv = biases
    scale = DK ** -0.5
    wq = wq * scale
    bq = bq * scale

    wqT = np.ascontiguousarray(wq.T).astype(NPBF16)
    wkT = np.ascontiguousarray(wk.T).astype(NPBF16)
    wvT = np.ascontiguousarray(wv.T).astype(NPBF16)
    woT = np.ascontiguousarray(out_w.T).astype(NPBF16)

    G = np.zeros((128, 8), np.float32)
    G[np.arange(128), np.arange(128) // GSZ] = 1.0
    GT = np.ascontiguousarray(G.T)
    ZPAD = np.zeros((64, S), NPBF16)

    has_bqk = bool(np.any(bq) or np.any(bk))
    has_bv = bool(np.any(bv))
    has_outb = bool(np.any(out_b))

    bqk = np.zeros((128, 8), np.float32)
    bqk[:, 0:4] = bq.reshape(4, 128).T
    bqk[:, 4:8] = bk.reshape(4, 128).T
    outb128 = np.ascontiguousarray(out_b.reshape(4, 128).T)

    nc = _build(has_bqk, has_bv, has_outb)

    xr = x.reshape(B, C, S)
    in_maps = []
    for c in range(N_CORES):
        m = {
            "x": np.ascontiguousarray(xr[c * BL : (c + 1) * BL]),
            "wqt": wqT,
            "wkt": wkT,
            "wvt": wvT,
            "wot": woT,
            "gmat": G,
            "gtmat": GT,
            "zpad": ZPAD,
        }
        if has_bqk:
            m["bqk"] = bqk
        if has_bv:
            m["bv"] = np.ascontiguousarray(bv.reshape(1, C)).astype(NPBF16)
        if has_outb:
            m["outb"] = outb128
        in_maps.append(m)

    # guard: bass_utils imports antenv.axon_hooks when tracing is requested
    # (e.g. via BASS_TRACE env); provide a no-op module if the image lacks it.
    try:
        import antenv.axon_hooks  # noqa: F401
    except ImportError:
        import sys
        import types

        import antenv

        _m = types.ModuleType("antenv.axon_hooks")
        _m._hook = None
        _m.set_axon_ntff_profile_hook = lambda h: setattr(_m, "_hook", h)
        _m.get_axon_ntff_profile_hook = lambda: _m._hook
        sys.modules["antenv.axon_hooks"] = _m
        antenv.axon_hooks = _m

    res = None
    for attempt in range(3):
        try:
            res = run_bass_kernel_spmd(
                nc, in_maps, core_ids=list(range(N_CORES)), trace=TRACE
            )
            break
        except Exception:
            # transient NRT_EXEC_UNIT_UNRECOVERABLE-style device hiccups
            # clear on retry; re-raise on the final attempt
            if attempt == 2:
                raise
    LAST["exec_time_ns"] = res.exec_time_ns
    LAST["mean_exec_time_ns"] = res.mean_exec_time_ns
    LAST["result"] = res

    out = np.concatenate([res.results[c]["out"] for c in range(N_CORES)], axis=0)
    return np.ascontiguousarray(out.reshape(B, C, H, W).astype(np.float32))
